# revision 14
# baseline (speedup 1.0000x reference)
"""Trainium2 Bass kernel for MultiHeadSelfAttention (GroupNorm + QKV + attention + proj + residual).

Problem shape (hardcoded): x [8, 512, 32, 32] fp32, 8 heads, 32 groups.
Sharding: data-parallel over batch B=8 across the 8 NeuronCores (one batch per core).

Per-core pipeline (T = 32*32 = 1024 positions, C = 512 channels, ch = 64 per head):
  1. GroupNorm(32) over [C, T]: per-channel bn_stats, group-combine via tiny PE
     matmuls with a group-indicator matrix, rsqrt via ACT ln/exp, affine fold.
  2. qkv = qkv_w @ h + b computed in two custom layouts (host-reordered weights):
       - q,k: [1024, T] with head-pair packing (k-pair tile, q-pair tile per pair)
       - v:   transposed directly, vT [T, 512], with a ones-rider column per head
  3. Per head: wT[s,t] = k^T q on PE (no transposes needed in this orientation),
     exp on ACT (softmax max-subtraction skipped: logits are O(1) by construction),
     AV matmul with ones-rider to get softmax sums for free, reciprocal +
     gpsimd partition_broadcast + DVE mul to normalize.
  4. proj matmul + bias + residual fused via the AFFINE_THEN_ADD custom DVE op.

All matmuls run in float32r (full PE rate, ~1.5e-4 rms rel err).
"""

import numpy as np

import concourse.bass as bass
import concourse.bacc as bacc
import concourse.tile as tile
import concourse.mybir as mybir
from concourse import library_config
from concourse.bass_utils import run_bass_kernel_spmd
from concourse.dve_ops import AFFINE_THEN_ADD

B, C, HS, WS = 8, 512, 32, 32
T = HS * WS            # 1024
H = 8                  # heads
CH = C // H            # 64
G = 32                 # groups
CPG = C // G           # 16 channels per group
EPS = 1e-5
NCHUNK = C // 128      # 4 channel chunks
NT = T // 128          # 8 sequence tiles
NB = T // 512          # 2 psum banks over T
F32 = mybir.dt.float32
F32R = mybir.dt.float32r

_CACHE = {}
_DEBUG = False


def _orig_row(kind, h, i):
    # row in qkv_w for head h, kind q/k/v, within-head index i
    off = {"q": 0, "k": CH, "v": 2 * CH}[kind]
    return 192 * h + off + i


def _host_weights(gn_w, gn_b, qkv_w, qkv_b, proj_w, proj_b):
    scale2 = 1.0 / np.sqrt(CH)  # ch**-0.25 applied to both q and k -> fold into k
    # qk weights: col layout m-tile 2p = [k_h0 | k_h1], m-tile 2p+1 = [q_h0 | q_h1]
    rows = np.zeros(2 * C, dtype=np.int64)
    colscale = np.ones(2 * C, dtype=np.float32)
    for p in range(H // 2):
        for slot in range(2):
            h = 2 * p + slot
            for i in range(CH):
                col_k = (2 * p) * 128 + slot * CH + i
                rows[col_k] = _orig_row("k", h, i)
                colscale[col_k] = scale2
                col_q = (2 * p + 1) * 128 + slot * CH + i
                rows[col_q] = _orig_row("q", h, i)
    wqk = (qkv_w[rows, :] * colscale[:, None]).T.copy()      # [512, 1024]
    wqk_t = np.ascontiguousarray(
        wqk.reshape(NCHUNK, 128, 2 * C)).astype(np.float32)  # [4, 128, 1024]
    bqk = (qkv_b[rows] * colscale).reshape(8, 128).T.copy()  # [128, 8]

    vrows = np.array([_orig_row("v", h, i) for h in range(H) for i in range(CH)])
    wv = qkv_w[vrows, :].T.copy()                            # [512, 512] (c, c_v)
    wv_t = np.ascontiguousarray(wv.reshape(NCHUNK, 128, C)).astype(np.float32)
    bv = qkv_b[vrows].reshape(1, C).astype(np.float32)

    wproj = proj_w.T.copy()                                  # [512(c), 512(o)]
    wproj_t = np.ascontiguousarray(wproj.reshape(NCHUNK, 128, C)).astype(np.float32)
    bproj = proj_b.reshape(NCHUNK, 128).T.copy()             # [128, 4]

    gnw = gn_w.reshape(NCHUNK, 128).T.copy()                 # [128, 4]
    gnb = gn_b.reshape(NCHUNK, 128).T.copy()

    riderpad = np.zeros((128, H, CH), dtype=np.float32)
    riderpad[:, :, 0] = 1.0
    g_all = np.zeros((128, 128), dtype=np.float32)           # [u, 32k+g] = 1/16
    gt_all = np.zeros((32, 512), dtype=np.float32)           # [g, 128k+u] = 1
    for k in range(NCHUNK):
        for u in range(128):
            g = 8 * k + u // CPG
            g_all[u, 32 * k + g] = 1.0 / CPG
            gt_all[g, 128 * k + u] = 1.0
    return {
        "wqk": wqk_t, "bqk": bqk, "wv": wv_t, "bv": bv,
        "wproj": wproj_t, "bproj": bproj, "gnw": gnw, "gnb": gnb,
        "g_all": g_all, "gt_all": gt_all,
        "riderpad": riderpad,
        "ones1": np.ones((1, 128), dtype=np.float32),
    }


def _build_program(n_reps=1):
    nc = bacc.Bacc("TRN2", target_bir_lowering=False, debug=False, num_devices=8)
    dt_in = [
        ("x", [C, T], F32), ("wqk", [NCHUNK, 128, 2 * C], F32R),
        ("bqk", [128, 8], F32), ("wv", [NCHUNK, 128, C], F32R),
        ("bv", [1, C], F32R), ("wproj", [NCHUNK, 128, C], F32R),
        ("bproj", [128, NCHUNK], F32), ("gnw", [128, NCHUNK], F32),
        ("gnb", [128, NCHUNK], F32), ("g_all", [128, 128], F32R),
        ("gt_all", [32, 512], F32R), ("riderpad", [128, H, CH], F32R),
        ("ones1", [1, 128], F32R),
    ]
    d = {name: nc.dram_tensor(name, shape, dt, kind="ExternalInput").ap()
         for name, shape, dt in dt_in}
    out_d = nc.dram_tensor("out", [C, T], F32, kind="ExternalOutput").ap()
    if _DEBUG:
        a_o = nc.dram_tensor("a_o", [C, T], F32R, kind="ExternalOutput").ap()
        rec_o = nc.dram_tensor("rec_o", [H, T], F32, kind="ExternalOutput").ap()
        rb_o = nc.dram_tensor("rb_o", [H, T], F32, kind="ExternalOutput").ap()

    with tile.TileContext(nc) as tc:
        with (
            tc.tile_pool(name="singles", bufs=1) as singles,
            tc.tile_pool(name="small", bufs=10) as small,
            tc.tile_pool(name="qkp", bufs=4) as qkp,
            tc.tile_pool(name="ewp", bufs=10) as ewp,
            tc.tile_pool(name="recp", bufs=3) as recp,
            tc.tile_pool(name="rbp", bufs=3) as rbp,
            tc.tile_pool(name="outp", bufs=2) as outp,
            tc.tile_pool(name="pA", bufs=3, space="PSUM") as pA,
            tc.tile_pool(name="pB", bufs=2, space="PSUM") as pB,
        ):
            nc.gpsimd.load_library(library_config.attn)

            # ---- load constants / weights ----
            wqk_sb = []
            for k in range(NCHUNK):
                t_ = singles.tile([128, 2 * C], F32R, tag=f"wqk{k}")
                nc.sync.dma_start(t_[:], d["wqk"][k])
                wqk_sb.append(t_)
            wv_sb = []
            for k in range(NCHUNK):
                t_ = singles.tile([128, C], F32R, tag=f"wv{k}")
                nc.sync.dma_start(t_[:], d["wv"][k])
                wv_sb.append(t_)
            wproj_sb = []
            for k in range(NCHUNK):
                t_ = singles.tile([128, C], F32R, tag=f"wproj{k}")
                nc.sync.dma_start(t_[:], d["wproj"][k])
                wproj_sb.append(t_)
            g_sb = singles.tile([128, 128], F32R, tag="g_all")
            nc.sync.dma_start(g_sb[:], d["g_all"][:])
            gt_sb = singles.tile([32, 512], F32R, tag="gt_all")
            nc.sync.dma_start(gt_sb[:], d["gt_all"][:])
            ones1_sb = singles.tile([1, 128], F32R, tag="ones1")
            nc.sync.dma_start(ones1_sb[:], d["ones1"][:])
            bv_sb = singles.tile([1, C], F32R, tag="bv")
            nc.sync.dma_start(bv_sb[:], d["bv"][:])
            bqk_sb = singles.tile([128, 8], F32, tag="bqk")
            nc.sync.dma_start(bqk_sb[:], d["bqk"][:])
            bproj_sb = singles.tile([128, NCHUNK], F32, tag="bproj")
            nc.sync.dma_start(bproj_sb[:], d["bproj"][:])
            gnw_sb = singles.tile([128, NCHUNK], F32, tag="gnw")
            nc.sync.dma_start(gnw_sb[:], d["gnw"][:])
            gnb_sb = singles.tile([128, NCHUNK], F32, tag="gnb")
            nc.sync.dma_start(gnb_sb[:], d["gnb"][:])
            eps_t = singles.tile([32, 1], F32, tag="eps")
            nc.vector.memset(eps_t[:], EPS)

            x_sb = []
            for k in range(NCHUNK):
                t_ = singles.tile([128, T], F32, tag=f"x{k}")
                nc.sync.dma_start(t_[:], d["x"][128 * k:128 * (k + 1), :])
                x_sb.append(t_)

            for rep in range(n_reps):
                sfx = f"r{rep}"
                # ================= GroupNorm =================
                h_sb = []
                psum_gs = pB.tile([32, 2], F32, tag="pB")
                stats_list = []
                for k in range(NCHUNK):
                    st6 = small.tile([128, 2, 6], F32, tag="small")
                    nc.vector.bn_stats(st6[:, 0, :], x_sb[k][:, 0:512])
                    nc.vector.bn_stats(st6[:, 1, :], x_sb[k][:, 512:1024])
                    mv = small.tile([128, 2], F32, tag="small")
                    nc.vector.bn_aggr(mv[:], st6[:])
                    m2 = small.tile([128, 1], F32, tag="small")
                    nc.vector.tensor_mul(m2[:], mv[:, 0:1], mv[:, 0:1])
                    stats = small.tile([128, 2], F32R, tag="small")
                    nc.vector.tensor_copy(stats[:, 0:1], mv[:, 0:1])
                    nc.vector.tensor_add(stats[:, 1:2], mv[:, 1:2], m2[:])
                    stats_list.append(stats)
                for k in range(NCHUNK):
                    nc.tensor.matmul(psum_gs[:], g_sb[:, 32 * k:32 * (k + 1)],
                                     stats_list[k][:], start=(k == 0), stop=(k == 3))
                gsb = small.tile([32, 2], F32, tag="small")
                nc.vector.tensor_copy(gsb[:], psum_gs[:])
                mu2 = small.tile([32, 1], F32, tag="small")
                nc.vector.tensor_mul(mu2[:], gsb[:, 0:1], gsb[:, 0:1])
                varg = small.tile([32, 1], F32, tag="small")
                nc.vector.tensor_sub(varg[:], gsb[:, 1:2], mu2[:])
                lnv = small.tile([32, 1], F32, tag="small")
                nc.scalar.activation(lnv[:], varg[:], mybir.ActivationFunctionType.Ln,
                                     bias=eps_t[:], scale=1.0)
                rstd = small.tile([32, 1], F32, tag="small")
                nc.scalar.activation(rstd[:], lnv[:], mybir.ActivationFunctionType.Exp,
                                     scale=-0.5)
                grp = small.tile([32, 2], F32R, tag="small")
                nc.vector.tensor_copy(grp[:, 0:1], gsb[:, 0:1])
                nc.vector.tensor_copy(grp[:, 1:2], rstd[:])
                for k in range(NCHUNK):
                    psum_pc = pB.tile([128, 2], F32, tag="pB")
                    nc.tensor.matmul(psum_pc[:], gt_sb[:, 128 * k:128 * (k + 1)],
                                     grp[:], start=True, stop=True)
                    s_c = small.tile([128, 1], F32, tag="small")
                    nc.vector.tensor_mul(s_c[:], psum_pc[:, 1:2], gnw_sb[:, k:k + 1])
                    t1 = small.tile([128, 1], F32, tag="small")
                    nc.vector.tensor_mul(t1[:], psum_pc[:, 0:1], s_c[:])
                    b_c = small.tile([128, 1], F32, tag="small")
                    nc.vector.tensor_sub(b_c[:], gnb_sb[:, k:k + 1], t1[:])
                    ht = singles.tile([128, T], F32R, tag=f"h{k}")
                    nc.vector.tensor_scalar(
                        out=ht[:], in0=x_sb[k][:], scalar1=s_c[:], scalar2=b_c[:],
                        op0=mybir.AluOpType.mult, op1=mybir.AluOpType.add)
                    h_sb.append(ht)

                # ================= vT = (h^T @ Wv)^T with bias rider ========
                vt_sb = []
                for mt in range(NT):
                    pv = pB.tile([128, C], F32, tag="pB")
                    for k in range(NCHUNK):
                        nc.tensor.matmul(pv[:], h_sb[k][:, 128 * mt:128 * (mt + 1)],
                                         wv_sb[k][:], start=(k == 0), stop=False)
                    nc.tensor.matmul(pv[:], ones1_sb[:], bv_sb[:],
                                     start=False, stop=True)
                    vt = singles.tile([128, H, 2 * CH], F32R, tag=f"vt{mt}")
                    nc.sync.dma_start(vt[:, :, 0:CH], d["riderpad"][:])
                    nc.vector.tensor_copy(
                        vt[:, :, CH:2 * CH], pv[:].rearrange("p (h c) -> p h c", h=H))
                    vt_sb.append(vt)

                # ================= q,k tiles =================
                def make_qk(m):
                    pq = pA.tile([128, T], F32, tag="pA")
                    for nb in range(NB):
                        for k in range(NCHUNK):
                            nc.tensor.matmul(
                                pq[:, 512 * nb:512 * (nb + 1)],
                                wqk_sb[k][:, 128 * m:128 * (m + 1)],
                                h_sb[k][:, 512 * nb:512 * (nb + 1)],
                                start=(k == 0), stop=(k == 3))
                    qk = qkp.tile([128, T], F32R, tag="qk")
                    nc.vector.tensor_scalar(
                        out=qk[:], in0=pq[:], scalar1=bqk_sb[:, m:m + 1], scalar2=None,
                        op0=mybir.AluOpType.add)
                    return qk

                # ================= attention =================
                a_sb = [singles.tile([128, T], F32R, tag=f"a{p}",
                                     name=f"a{p}{sfx}")
                        for p in range(NCHUNK)]
                for p in range(H // 2):
                    ktile = make_qk(2 * p)
                    qtile = make_qk(2 * p + 1)
                    for slot in range(2):
                        h = 2 * p + slot
                        lo, hi = CH * slot, CH * (slot + 1)
                        ew_tiles = []
                        for st in range(NT):
                            ew = ewp.tile([128, T], F32R, tag="ew")
                            for nb in range(NB):
                                pw = pB.tile([128, 512], F32, tag="pB")
                                nc.tensor.matmul(
                                    pw[:], ktile[lo:hi, 128 * st:128 * (st + 1)],
                                    qtile[lo:hi, 512 * nb:512 * (nb + 1)],
                                    start=True, stop=True)
                                nc.scalar.activation(
                                    ew[:, 512 * nb:512 * (nb + 1)], pw[:],
                                    mybir.ActivationFunctionType.Exp)
                            ew_tiles.append(ew)
                        pa = pA.tile([128, T], F32, tag="pA")
                        for nb in range(NB):
                            for sc in range(NT):
                                nc.tensor.matmul(
                                    pa[:, 512 * nb:512 * (nb + 1)],
                                    vt_sb[sc][:, h, :],
                                    ew_tiles[sc][:, 512 * nb:512 * (nb + 1)],
                                    start=(sc == 0), stop=(sc == NT - 1))
                        rec = recp.tile([1, T], F32, tag="rec")
                        nc.vector.reciprocal_approx_fast(rec[:], pa[0:1, :])
                        rb = rbp.tile([CH, T], F32, tag="rb")
                        nc.gpsimd.partition_broadcast(rb[:], rec[:])
                        nc.vector.tensor_mul(a_sb[p][lo:hi, :], pa[CH:2 * CH, :], rb[:])
                        if _DEBUG and rep == n_reps - 1:
                            nc.sync.dma_start(rec_o[h:h + 1, :], rec[:])
                            nc.sync.dma_start(rb_o[h:h + 1, :], rb[0:1, :])

                if _DEBUG and rep == n_reps - 1:
                    for p in range(NCHUNK):
                        nc.sync.dma_start(a_o[128 * p:128 * (p + 1), :], a_sb[p][:])

                # ================= proj + residual =================
                for m in range(NCHUNK):
                    po = pA.tile([128, T], F32, tag="pA")
                    for nb in range(NB):
                        for k in range(NCHUNK):
                            nc.tensor.matmul(
                                po[:, 512 * nb:512 * (nb + 1)],
                                wproj_sb[k][:, 128 * m:128 * (m + 1)],
                                a_sb[k][:, 512 * nb:512 * (nb + 1)],
                                start=(k == 0), stop=(k == 3))
                    ot = outp.tile([128, T], F32, tag="out")
                    nc.vector._custom_dve(
                        AFFINE_THEN_ADD, out=ot[:], in0=po[:], in1=x_sb[m][:],
                        s0=1.0, s1=bproj_sb[:, m:m + 1])
                    if rep == n_reps - 1:
                        nc.sync.dma_start(out_d[128 * m:128 * (m + 1), :], ot[:])

    nc.compile()
    return nc


def _get_program(n_reps=1):
    key = ("prog", n_reps)
    if key not in _CACHE:
        _CACHE[key] = _build_program(n_reps)
    return _CACHE[key]


def kernel(x, gn_w, gn_b, qkv_w, qkv_b, proj_w, proj_b, _n_reps=1):
    x = np.asarray(x, dtype=np.float32)
    hw = _host_weights(np.asarray(gn_w, np.float32), np.asarray(gn_b, np.float32),
                       np.asarray(qkv_w, np.float32), np.asarray(qkv_b, np.float32),
                       np.asarray(proj_w, np.float32), np.asarray(proj_b, np.float32))
    xr = np.ascontiguousarray(x.reshape(B, C, T))
    nc = _get_program(_n_reps)
    in_maps = [dict(hw, x=xr[b]) for b in range(B)]
    res = run_bass_kernel_spmd(nc, in_maps, core_ids=list(range(B)))
    out = np.stack([res.results[b]["out"] for b in range(B)])
    return out.reshape(B, C, HS, WS).astype(np.float32)


# revision 54
# speedup vs baseline: 591.3218x; 591.3218x over previous
"""Trainium2 Bass kernel for MultiHeadSelfAttention (GroupNorm + QKV + attention + proj + residual).

Problem shape (hardcoded): x [8, 512, 32, 32] fp32, 8 heads, 32 groups.
Sharding: data-parallel over batch B=8 across the 8 NeuronCores (one batch per core).

Per-core pipeline (T = 32*32 = 1024 positions, C = 512 channels, ch = 64 per head):
  1. GroupNorm(32) over [C, T]: per-channel bn_stats, group-combine via tiny PE
     matmuls with a group-indicator matrix, rsqrt via ACT ln/exp, affine fold.
  2. qkv = qkv_w @ h + b computed in two custom layouts (host-reordered weights):
       - q,k: [1024, T] with head-pair packing (k-pair tile, q-pair tile per pair)
       - v:   transposed directly, vT [T, 512], with a ones-rider column per head
  3. Per head: wT[s,t] = k^T q on PE (no transposes needed in this orientation),
     exp on ACT (softmax max-subtraction skipped: logits are O(1) by construction),
     AV matmul with ones-rider to get softmax sums for free, reciprocal +
     gpsimd partition_broadcast + DVE mul to normalize.
  4. proj matmul + bias + residual fused via the AFFINE_THEN_ADD custom DVE op.

All matmuls run in float32r (full PE rate, ~1.5e-4 rms rel err).
"""

import ml_dtypes
import numpy as np

import concourse.bass as bass
import concourse.bacc as bacc
import concourse.tile as tile
import concourse.mybir as mybir
from concourse import library_config
from concourse.bass_utils import run_bass_kernel_spmd
from concourse.dve_ops import AFFINE_THEN_ADD

B, C, HS, WS = 8, 512, 32, 32
T = HS * WS            # 1024
H = 8                  # heads
CH = C // H            # 64
G = 32                 # groups
CPG = C // G           # 16 channels per group
EPS = 1e-5
NCHUNK = C // 128      # 4 channel chunks
NT = T // 128          # 8 sequence tiles
NB = T // 512          # 2 psum banks over T
F32 = mybir.dt.float32
F32R = mybir.dt.float32r
BF16 = mybir.dt.bfloat16

_CACHE = {}
_DEBUG = False


def _orig_row(kind, h, i):
    # row in qkv_w for head h, kind q/k/v, within-head index i
    off = {"q": 0, "k": CH, "v": 2 * CH}[kind]
    return 192 * h + off + i


def _host_weights(gn_w, gn_b, qkv_w, qkv_b, proj_w, proj_b):
    scale2 = 1.0 / np.sqrt(CH)  # ch**-0.25 applied to both q and k -> fold into k
    # qk weights: col layout m-tile 2p = [k_h0 | k_h1], m-tile 2p+1 = [q_h0 | q_h1]
    rows = np.zeros(2 * C, dtype=np.int64)
    colscale = np.ones(2 * C, dtype=np.float32)
    for p in range(H // 2):
        for slot in range(2):
            h = 2 * p + slot
            for i in range(CH):
                col_k = (2 * p) * 128 + slot * CH + i
                rows[col_k] = _orig_row("k", h, i)
                colscale[col_k] = scale2
                col_q = (2 * p + 1) * 128 + slot * CH + i
                rows[col_q] = _orig_row("q", h, i)
    wqk = (qkv_w[rows, :] * colscale[:, None]).T.copy()      # [512, 1024]
    wqk_t = np.ascontiguousarray(
        wqk.reshape(NCHUNK, 128, 2 * C)).astype(np.float32)  # [4, 128, 1024]
    bqk = (qkv_b[rows] * colscale).reshape(8, 128).T.copy()  # [128, 8]

    vrows = np.array([_orig_row("v", h, i) for h in range(H) for i in range(CH)])
    wv = qkv_w[vrows, :].T.copy()                            # [512, 512] (c, c_v)
    wv_t = np.ascontiguousarray(wv.reshape(NCHUNK, 128, C)).astype(np.float32)
    bv = qkv_b[vrows].reshape(1, C).astype(np.float32)

    wproj = proj_w.T.copy()                                  # [512(c), 512(o)]
    wproj_t = np.ascontiguousarray(wproj.reshape(NCHUNK, 128, C)).astype(np.float32)
    bproj = proj_b.reshape(NCHUNK, 128).T.copy()             # [128, 4]

    gnw = gn_w.reshape(NCHUNK, 128).T.copy()                 # [128, 4]
    gnb = gn_b.reshape(NCHUNK, 128).T.copy()

    riderpad = np.zeros((128, H, CH), dtype=np.float32)
    riderpad[:, :, 0] = 1.0
    riderpad = riderpad.astype(ml_dtypes.bfloat16)
    g_all = np.zeros((128, 128), dtype=np.float32)           # [u, 32k+g] = 1/16
    gt_all = np.zeros((32, 512), dtype=np.float32)           # [g, 128k+u] = 1
    for k in range(NCHUNK):
        for u in range(128):
            g = 8 * k + u // CPG
            g_all[u, 32 * k + g] = 1.0 / CPG
            gt_all[g, 128 * k + u] = 1.0
    return {
        "wqk": wqk_t, "bqk": bqk, "wv": wv_t, "bv": bv,
        "wproj": wproj_t, "bproj": bproj, "gnw": gnw, "gnb": gnb,
        "g_all": g_all, "gt_all": gt_all,
        "riderpad": riderpad,
        "ones1": np.ones((1, 128), dtype=np.float32),
    }


def _build_program(n_reps=1, pa_bufs=2, pb_bufs=4, ew_bufs=18, qk_bufs=4, big_exp=False, pw_bufs=2, qk_act=(0, 1)):
    nc = bacc.Bacc("TRN2", target_bir_lowering=False, debug=False, num_devices=8)
    dt_in = [
        ("x", [C, T], F32), ("wqk", [NCHUNK, 128, 2 * C], F32R),
        ("bqk", [128, 8], F32), ("wv", [NCHUNK, 128, C], F32R),
        ("bv", [1, C], F32R), ("wproj", [NCHUNK, 128, C], F32R),
        ("bproj", [128, NCHUNK], F32), ("gnw", [128, NCHUNK], F32),
        ("gnb", [128, NCHUNK], F32), ("g_all", [128, 128], F32R),
        ("gt_all", [32, 512], F32R), ("riderpad", [128, H, CH], BF16),
        ("ones1", [1, 128], F32R),
    ]
    d = {name: nc.dram_tensor(name, shape, dt, kind="ExternalInput").ap()
         for name, shape, dt in dt_in}
    out_d = nc.dram_tensor("out", [C, T], F32, kind="ExternalOutput").ap()
    if _DEBUG:
        a_o = nc.dram_tensor("a_o", [C, T], F32R, kind="ExternalOutput").ap()
        rec_o = nc.dram_tensor("rec_o", [H, T], F32, kind="ExternalOutput").ap()
        rb_o = nc.dram_tensor("rb_o", [H, T], F32, kind="ExternalOutput").ap()

    with tile.TileContext(nc) as tc:
        with (
            tc.tile_pool(name="singles", bufs=1) as singles,
            tc.tile_pool(name="small", bufs=10) as small,
            tc.tile_pool(name="qkp", bufs=qk_bufs) as qkp,
            tc.tile_pool(name="ewp", bufs=ew_bufs) as ewp,
            tc.tile_pool(name="recp", bufs=3) as recp,
            tc.tile_pool(name="rbp", bufs=3) as rbp,
            tc.tile_pool(name="outp", bufs=2) as outp,
            tc.tile_pool(name="pA", bufs=pa_bufs, space="PSUM") as pA,
            tc.tile_pool(name="pB", bufs=(pw_bufs if big_exp else pb_bufs),
                         space="PSUM") as pB,
        ):
            nc.gpsimd.load_library(library_config.attn)

            # ---- load inputs: x + GN-critical consts first (sync queue),
            # ---- bulk weights for later phases on the gpsimd queue ----
            x_sb = []
            for k in range(NCHUNK):
                t_ = singles.tile([128, T], F32, tag=f"x{k}")
                nc.sync.dma_start(t_[:], d["x"][128 * k:128 * (k + 1), :])
                x_sb.append(t_)
            g_sb = singles.tile([128, 128], F32R, tag="g_all")
            nc.sync.dma_start(g_sb[:], d["g_all"][:])
            gt_sb = singles.tile([32, 512], F32R, tag="gt_all")
            nc.sync.dma_start(gt_sb[:], d["gt_all"][:])
            gnw_sb = singles.tile([128, NCHUNK], F32, tag="gnw")
            nc.sync.dma_start(gnw_sb[:], d["gnw"][:])
            gnb_sb = singles.tile([128, NCHUNK], F32, tag="gnb")
            nc.sync.dma_start(gnb_sb[:], d["gnb"][:])
            bqk_sb = singles.tile([128, 8], F32, tag="bqk")
            nc.sync.dma_start(bqk_sb[:], d["bqk"][:])
            wqk_sb = []
            for k in range(NCHUNK):
                t_ = singles.tile([128, 2 * C], F32R, tag=f"wqk{k}")
                nc.sync.dma_start(t_[:], d["wqk"][k])
                wqk_sb.append(t_)
            eps_t = singles.tile([32, 1], F32, tag="eps")
            nc.vector.memset(eps_t[:], EPS)
            wv_sb = []
            for k in range(NCHUNK):
                t_ = singles.tile([128, C], F32R, tag=f"wv{k}")
                nc.gpsimd.dma_start(t_[:], d["wv"][k])
                wv_sb.append(t_)
            wproj_sb = []
            for k in range(NCHUNK):
                t_ = singles.tile([128, C], F32R, tag=f"wproj{k}")
                nc.gpsimd.dma_start(t_[:], d["wproj"][k])
                wproj_sb.append(t_)
            ones1_sb = singles.tile([1, 128], F32R, tag="ones1")
            nc.gpsimd.dma_start(ones1_sb[:], d["ones1"][:])
            bv_sb = singles.tile([1, C], F32R, tag="bv")
            nc.gpsimd.dma_start(bv_sb[:], d["bv"][:])
            bproj_sb = singles.tile([128, NCHUNK], F32, tag="bproj")
            nc.gpsimd.dma_start(bproj_sb[:], d["bproj"][:])

            for rep in range(n_reps):
                sfx = f"r{rep}"
                # ================= GroupNorm =================
                h_sb = []
                psum_gs = pB.tile([32, 2], F32, tag="pB", name="psum_gs")
                stats_list = []
                for k in range(NCHUNK):
                    st6 = small.tile([128, 2, 6], F32, tag="small")
                    nc.vector.bn_stats(st6[:, 0, :], x_sb[k][:, 0:512])
                    nc.vector.bn_stats(st6[:, 1, :], x_sb[k][:, 512:1024])
                    mv = small.tile([128, 2], F32, tag="small")
                    nc.vector.bn_aggr(mv[:], st6[:])
                    m2 = small.tile([128, 1], F32, tag="small")
                    nc.vector.tensor_mul(m2[:], mv[:, 0:1], mv[:, 0:1])
                    stats = small.tile([128, 2], F32R, tag="small")
                    nc.vector.tensor_copy(stats[:, 0:1], mv[:, 0:1])
                    nc.vector.tensor_add(stats[:, 1:2], mv[:, 1:2], m2[:])
                    stats_list.append(stats)
                for k in range(NCHUNK):
                    nc.tensor.matmul(psum_gs[:], g_sb[:, 32 * k:32 * (k + 1)],
                                     stats_list[k][:], start=(k == 0), stop=(k == 3))
                gsb = small.tile([32, 2], F32, tag="small")
                nc.vector.tensor_copy(gsb[:], psum_gs[:])
                mu2 = small.tile([32, 1], F32, tag="small")
                nc.vector.tensor_mul(mu2[:], gsb[:, 0:1], gsb[:, 0:1])
                varg = small.tile([32, 1], F32, tag="small")
                nc.vector.tensor_sub(varg[:], gsb[:, 1:2], mu2[:])
                lnv = small.tile([32, 1], F32, tag="small")
                nc.scalar.activation(lnv[:], varg[:], mybir.ActivationFunctionType.Ln,
                                     bias=eps_t[:], scale=1.0)
                rstd = small.tile([32, 1], F32, tag="small")
                nc.scalar.activation(rstd[:], lnv[:], mybir.ActivationFunctionType.Exp,
                                     scale=-0.5)
                grp = small.tile([32, 2], F32R, tag="small")
                nc.vector.tensor_copy(grp[:, 0:1], gsb[:, 0:1])
                nc.vector.tensor_copy(grp[:, 1:2], rstd[:])
                for k in range(NCHUNK):
                    psum_pc = pB.tile([128, 2], F32, tag="pB", name="psum_pc")
                    nc.tensor.matmul(psum_pc[:], gt_sb[:, 128 * k:128 * (k + 1)],
                                     grp[:], start=True, stop=True)
                    s_c = small.tile([128, 1], F32, tag="small")
                    nc.vector.tensor_mul(s_c[:], psum_pc[:, 1:2], gnw_sb[:, k:k + 1])
                    t1 = small.tile([128, 1], F32, tag="small")
                    nc.vector.tensor_mul(t1[:], psum_pc[:, 0:1], s_c[:])
                    b_c = small.tile([128, 1], F32, tag="small")
                    nc.vector.tensor_sub(b_c[:], gnb_sb[:, k:k + 1], t1[:])
                    ht = singles.tile([128, T], F32R, tag=f"h{k}", name=f"h{k}")
                    nc.vector.tensor_scalar(
                        out=ht[:], in0=x_sb[k][:], scalar1=s_c[:], scalar2=b_c[:],
                        op0=mybir.AluOpType.mult, op1=mybir.AluOpType.add)
                    h_sb.append(ht)

                # ============ vT = (h^T @ Wv)^T with bias rider (lazy) ======
                def emit_vt_tile(mt):
                    pv = pB.tile([128, C], F32, tag="pB", name="pv")
                    for k in range(NCHUNK):
                        nc.tensor.matmul(pv[:],
                                         h_sb[k][:, 128 * mt:128 * (mt + 1)],
                                         wv_sb[k][:], start=(k == 0), stop=False)
                    nc.tensor.matmul(pv[:], ones1_sb[:], bv_sb[:],
                                     start=False, stop=True)
                    vt = singles.tile([128, H, 2 * CH], BF16, tag=f"vt{mt}",
                                      name=f"vt{mt}")
                    nc.sync.dma_start(vt[:, :, 0:CH], d["riderpad"][:])
                    nc.vector.tensor_copy(
                        vt[:, :, CH:2 * CH],
                        pv[:].rearrange("p (h c) -> p h c", h=H))
                    return vt

                # ================= q,k tiles =================
                def make_qk(m):
                    pq = pA.tile([128, T], F32, tag="pA")
                    for nb in range(NB):
                        for k in range(NCHUNK):
                            nc.tensor.matmul(
                                pq[:, 512 * nb:512 * (nb + 1)],
                                wqk_sb[k][:, 128 * m:128 * (m + 1)],
                                h_sb[k][:, 512 * nb:512 * (nb + 1)],
                                start=(k == 0), stop=(k == 3))
                    qk = qkp.tile([128, T], F32R, tag="qk")
                    if m in qk_act:
                        nc.scalar.activation(
                            qk[:], pq[:], mybir.ActivationFunctionType.Identity,
                            bias=bqk_sb[:, m:m + 1])
                    else:
                        nc.vector.tensor_scalar(
                            out=qk[:], in0=pq[:], scalar1=bqk_sb[:, m:m + 1],
                            scalar2=None, op0=mybir.AluOpType.add)
                    return qk

                # ================= attention =================
                a_sb = [singles.tile([128, T], F32R, tag=f"a{p}",
                                     name=f"a{p}{sfx}")
                        for p in range(NCHUNK)]
                vt_sb = None
                qk_tiles = {}

                def emit_qk_step(h, st, ew_h):
                    # 2 matmuls + 2 exps for head h, s-tile st (both t-banks)
                    p, slot = h // 2, h % 2
                    lo, hi = CH * slot, CH * (slot + 1)
                    ktile, qtile = qk_tiles[2 * p], qk_tiles[2 * p + 1]
                    for nb in range(NB):
                        pw = pB.tile([128, 512], F32, tag="pB", name="pw")
                        nc.tensor.matmul(
                            pw[:], ktile[lo:hi, 128 * st:128 * (st + 1)],
                            qtile[lo:hi, 512 * nb:512 * (nb + 1)],
                            start=True, stop=True)
                        nc.scalar.activation(
                            ew_h[st][:, 512 * nb:512 * (nb + 1)], pw[:],
                            mybir.ActivationFunctionType.Exp)

                def make_qk(m):
                    pq = pA.tile([128, T], F32, tag="pA", name="pq")
                    for nb in range(NB):
                        for k in range(NCHUNK):
                            nc.tensor.matmul(
                                pq[:, 512 * nb:512 * (nb + 1)],
                                wqk_sb[k][:, 128 * m:128 * (m + 1)],
                                h_sb[k][:, 512 * nb:512 * (nb + 1)],
                                start=(k == 0), stop=(k == 3))
                    qk = qkp.tile([128, T], F32R, tag="qk", name="qk")
                    if m in qk_act:
                        nc.scalar.activation(
                            qk[:], pq[:], mybir.ActivationFunctionType.Identity,
                            bias=bqk_sb[:, m:m + 1])
                    else:
                        nc.vector.tensor_scalar(
                            out=qk[:], in0=pq[:], scalar1=bqk_sb[:, m:m + 1],
                            scalar2=None, op0=mybir.AluOpType.add)
                    return qk

                def qk_spread_duty(m):
                    # generator of per-step emissions producing qk tile m
                    # (8 matmuls over 4 steps, then the bias-add)
                    pq = pA.tile([128, T], F32, tag="pA", name="pq")
                    qk = qkp.tile([128, T], F32R, tag="qk", name="qk")
                    qk_tiles[m] = qk
                    for nb in range(NB):
                        for k in range(0, NCHUNK, 2):
                            nc.tensor.matmul(
                                pq[:, 512 * nb:512 * (nb + 1)],
                                wqk_sb[k][:, 128 * m:128 * (m + 1)],
                                h_sb[k][:, 512 * nb:512 * (nb + 1)],
                                start=(k == 0), stop=False)
                            nc.tensor.matmul(
                                pq[:, 512 * nb:512 * (nb + 1)],
                                wqk_sb[k + 1][:, 128 * m:128 * (m + 1)],
                                h_sb[k + 1][:, 512 * nb:512 * (nb + 1)],
                                start=False, stop=(k + 1 == NCHUNK - 1))
                            yield
                    nc.vector.tensor_scalar(
                        out=qk[:], in0=pq[:], scalar1=bqk_sb[:, m:m + 1],
                        scalar2=None, op0=mybir.AluOpType.add)
                    yield

                def proj_partial_duty(p):
                    # pair p's proj contribution: 8 (m, nb) matmul+add units.
                    # p == 0 fuses the bias + residual init via the native
                    # scalar_tensor_tensor ((po + b) + x) — custom DVE ops
                    # mishandle AP offsets so they are avoided on slices.
                    for m in range(NCHUNK):
                        for nb in range(NB):
                            sl = slice(512 * nb, 512 * (nb + 1))
                            po = pB.tile([128, 512], F32, tag="pB", name="po")
                            nc.tensor.matmul(
                                po[:],
                                wproj_sb[p][:, 128 * m:128 * (m + 1)],
                                a_sb[p][:, sl],
                                start=True, stop=True)
                            if p == 0:
                                nc.vector.scalar_tensor_tensor(
                                    out=acc_sb[m][:, sl], in0=po[:],
                                    scalar=bproj_sb[:, m:m + 1],
                                    in1=x_sb[m][:, sl],
                                    op0=mybir.AluOpType.add,
                                    op1=mybir.AluOpType.add)
                            else:
                                nc.vector.tensor_add(
                                    acc_sb[m][:, sl], po[:],
                                    acc_sb[m][:, sl])
                            yield

                ew = {hh: [ewp.tile([128, T], BF16, tag="ew", name=f"ew{hh}")
                           for _ in range(NT)] for hh in range(H)}
                acc_sb = [singles.tile([128, T], F32, tag=f"acc{m}",
                                       name=f"acc{m}{sfx}")
                          for m in range(NCHUNK)]
                qk_tiles[0] = make_qk(0)
                # q tile of pair 0 produced bank-by-bank: QK(0) over t-bank 0
                # starts while q's bank 1 is still in the matmul queue (the
                # k-tile windows span all of t, so k must be complete first)
                pq1 = pA.tile([128, T], F32, tag="pA", name="pq1")
                qk1 = qkp.tile([128, T], F32R, tag="qk", name="qk1")
                qk_tiles[1] = qk1
                for nb in range(NB):
                    sl = slice(512 * nb, 512 * (nb + 1))
                    for k in range(NCHUNK):
                        nc.tensor.matmul(
                            pq1[:, sl], wqk_sb[k][:, 128:256],
                            h_sb[k][:, sl], start=(k == 0), stop=(k == 3))
                    nc.vector.tensor_scalar(
                        out=qk1[:, sl], in0=pq1[:, sl], scalar1=bqk_sb[:, 1:2],
                        scalar2=None, op0=mybir.AluOpType.add)
                    ktile = qk_tiles[0]
                    for st in range(NT):
                        pw = pB.tile([128, 512], F32, tag="pB", name="pw")
                        nc.tensor.matmul(
                            pw[:], ktile[0:CH, 128 * st:128 * (st + 1)],
                            qk1[0:CH, sl], start=True, stop=True)
                        nc.scalar.activation(
                            ew[0][st][:, sl], pw[:],
                            mybir.ActivationFunctionType.Exp)
                vt_sb = [emit_vt_tile(0)]

                for h in range(H):
                    p, slot = h // 2, h % 2
                    lo, hi = CH * slot, CH * (slot + 1)
                    duties = []
                    if slot == 0 and h + 2 < H:
                        # produce next pair's q,k tiles during this head; they
                        # are consumed by QK steps starting at head h+1
                        duties.append(qk_spread_duty(h + 2))
                        duties.append(qk_spread_duty(h + 3))
                    if slot == 1 and p >= 1:
                        duties.append(proj_partial_duty(p - 1))
                    pa = pA.tile([128, T], F32, tag="pA", name="pa")
                    for st in range(NT):
                        if h + 1 < H:
                            emit_qk_step(h + 1, st, ew[h + 1])
                        if h == 0 and st + 1 < NT:
                            vt_sb.append(emit_vt_tile(st + 1))
                        advanced = 0
                        while duties and advanced < 2:
                            try:
                                next(duties[0])
                                advanced += 1
                            except StopIteration:
                                duties.pop(0)
                        for nb in range(NB):
                            nc.tensor.matmul(
                                pa[:, 512 * nb:512 * (nb + 1)],
                                vt_sb[st][:, h, :],
                                ew[h][st][:, 512 * nb:512 * (nb + 1)],
                                start=(st == 0), stop=(st == NT - 1))
                    for g in duties:
                        for _ in g:
                            pass
                    if h == H - 1:
                        for nb in range(NB):
                            sl = slice(512 * nb, 512 * (nb + 1))
                            rcb = recp.tile([1, 512], F32, tag="rcb",
                                            name="rcb")
                            nc.vector.reciprocal_approx_fast(
                                rcb[:], pa[0:1, sl])
                            rbb = rbp.tile([CH, 512], F32, tag="rbb",
                                           name="rbb")
                            nc.gpsimd.partition_broadcast(rbb[:], rcb[:])
                            nc.vector.tensor_mul(
                                a_sb[p][lo:hi, sl], pa[CH:2 * CH, sl], rbb[:])
                    else:
                        rec = recp.tile([1, T], F32, tag="rec")
                        nc.vector.reciprocal_approx_fast(rec[:], pa[0:1, :])
                        rb = rbp.tile([CH, T], F32, tag="rb")
                        nc.gpsimd.partition_broadcast(rb[:], rec[:])
                        nc.vector.tensor_mul(a_sb[p][lo:hi, :],
                                             pa[CH:2 * CH, :], rb[:])
                    rec = None
                    if _DEBUG and rep == n_reps - 1:
                        nc.sync.dma_start(rec_o[h:h + 1, :], rec[:])
                        nc.sync.dma_start(rb_o[h:h + 1, :], rb[0:1, :])

                if _DEBUG and rep == n_reps - 1:
                    for pp in range(NCHUNK):
                        nc.sync.dma_start(a_o[128 * pp:128 * (pp + 1), :],
                                          a_sb[pp][:])

                # ====== tail: pair-3 proj contribution + out ======
                for m in range(NCHUNK):
                    for nb in range(NB):
                        po = pB.tile([128, 512], F32, tag="pB", name="po")
                        nc.tensor.matmul(
                            po[:], wproj_sb[3][:, 128 * m:128 * (m + 1)],
                            a_sb[3][:, 512 * nb:512 * (nb + 1)],
                            start=True, stop=True)
                        ot_slice = acc_sb[m][:, 512 * nb:512 * (nb + 1)]
                        nc.vector.tensor_add(ot_slice, po[:], ot_slice)
                    if rep == n_reps - 1:
                        nc.sync.dma_start(out_d[128 * m:128 * (m + 1), :],
                                          acc_sb[m][:])

    nc.compile()
    return nc


def _get_program(n_reps=1):
    key = ("prog", n_reps)
    if key not in _CACHE:
        _CACHE[key] = _build_program(n_reps)
    return _CACHE[key]


def kernel(x, gn_w, gn_b, qkv_w, qkv_b, proj_w, proj_b, _n_reps=1):
    x = np.asarray(x, dtype=np.float32)
    hw = _host_weights(np.asarray(gn_w, np.float32), np.asarray(gn_b, np.float32),
                       np.asarray(qkv_w, np.float32), np.asarray(qkv_b, np.float32),
                       np.asarray(proj_w, np.float32), np.asarray(proj_b, np.float32))
    xr = np.ascontiguousarray(x.reshape(B, C, T))
    nc = _get_program(_n_reps)
    in_maps = [dict(hw, x=xr[b]) for b in range(B)]
    res = run_bass_kernel_spmd(nc, in_maps, core_ids=list(range(B)))
    out = np.stack([res.results[b]["out"] for b in range(B)])
    return out.reshape(B, C, HS, WS).astype(np.float32)
